# revision 6
# baseline (speedup 1.0000x reference)
"""Trainium2 Bass kernel for the 3-view attention-fusion pooling module (v3).

Computation (reference):
    t_k  = tanh(W @ x_k)                      (A=256, D=256), k = 1..3
    s_k  = h_n @ t_k                          (1, D)
    beta = softmax([s_1; s_2; s_3], axis=0)   (3, D)
    out  = beta[0]*x1 + beta[1]*x2 + beta[2]*x3   (N, D)

v3 restructure vs v2:
  * Host precomputes d2 = x2-x1, d3 = x3-x1 (bf16). Then
        u2 = W@x2 = W@x1 + W@d2   (two tiny [128,256] adds post-GEMM)
        out = x1 + beta2*d2 + beta3*d3   (4 elementwise ops, not 5)
    The GEMM streams (x1, d2, d3) so HBM traffic is unchanged.
  * Phase 2 runs in 512-col chunks, whole chunks assigned to DVE or
    GpSimd, with the beta operand a single [128, 2*512] tile (the
    512-periodicity along the free dim makes one operand serve every
    chunk). No full-width broadcast materialization.
  * Rows sharded across 8 cores; W fed per-core as W[:, shard].T; the
    (A, D) GEMM partials AllReduce-summed in bf16; everything downstream
    of the reduction is tiny and computed redundantly per core.
"""

import os
import sys

import numpy as np

for _p in ("/opt/trn_rl_repo", "/root/.axon_site/_ro/trn_rl_repo"):
    if _p not in sys.path:
        sys.path.append(_p)

import concourse.bacc as bacc
import concourse.tile as tile
from concourse import mybir
from concourse.bass_utils import run_bass_kernel_spmd

N_CORES = 8
N = 100000
D = 256          # feature dim
A = 256          # input_att
N_LOC = N // N_CORES   # 12500 rows per core
P = 125          # partitions per batch (matmul contraction chunk)
R = 10           # rows per partition per batch
NB = N_LOC // (P * R)  # 10 batches, all stashed in SBUF
FW = R * D       # free width of a batched SBUF tile
CH = 512         # phase-2 chunk width (multiple of D)
NCH = FW // CH

FP32 = mybir.dt.float32
BF16 = mybir.dt.bfloat16


def _emit_iteration(nc, tc, rep, xrs, wtr, outr, hn_sb, ones_sb, pdram,
                    n_cores, collective, phase2=True, matmul=True,
                    pool_batches=4, nq=3, r_=20, ident_sb=None, p2pe=False):
    NB = N_LOC // (P * r_)
    FW = r_ * D
    Tanh = mybir.ActivationFunctionType.Tanh
    Exp = mybir.ActivationFunctionType.Exp
    r = rep

    with (
        tc.tile_pool(name=f"pst1_{r}", bufs=N_LOC // (P * r_)) as pst1,
        tc.tile_pool(name=f"pst2_{r}", bufs=N_LOC // (P * r_)) as pst2,
        tc.tile_pool(name=f"pst3_{r}", bufs=N_LOC // (P * r_)) as pst3,
        tc.tile_pool(name=f"small_{r}", bufs=1) as small,
    ):
        stpools = (pst1, pst2, pst3)

        # ---------------- phase 1: u = W @ (x1, d2, d3) partials -----------
        # cc layout: column block (h*3 + v)*D holds u_v rows h*128..h*128+127
        stash = []
        cc_in = small.tile([128, 6 * D], BF16, name="cc_in", tag="cc_in")
        with (
            tc.tile_pool(name=f"pacc_{r}", bufs=1, space="PSUM") as pacc,
            tc.tile_pool(name=f"pw_{r}", bufs=3) as pw,
        ):
            uacc = [[pacc.tile([128, D], FP32, name=f"u{v}{h}",
                               tag=f"u{v}{h}")
                     for h in range(2)] for v in range(3)]
            engs = [nc.sync, nc.scalar, nc.gpsimd][:nq]
            wpb = r_ // R          # W chunks (of R rows) per x batch
            qi = 0
            for b in range(NB):
                xts = [stpools[v].tile([P, FW], BF16, name=f"xs{v}",
                                       tag="xs") for v in range(3)]
                stash.append(xts)
                wtiles = []
                if (b == 0 or b == NB - 1) and matmul:
                    # first batch: pieces let the GEMM start ~3us earlier;
                    # last batch: W first + x pieces shrink the post-load
                    # matmul tail to one piece's worth
                    for k in range(wpb):
                        wtile = pw.tile([P, R * A], BF16, name="w", tag="w")
                        engs[qi % nq].dma_start(wtile[:], wtr[b][k])
                        qi += 1
                        wtiles.append(wtile)
                    npc = 4
                    pc = FW // npc
                    for q in range(npc):
                        for v in range(3):
                            cs = slice(q * pc, (q + 1) * pc)
                            engs[qi % nq].dma_start(xts[v][:, cs],
                                                    xrs[v][b][:, cs])
                            qi += 1
                    for g in range(r_):
                        last = (b == NB - 1 and g == r_ - 1)
                        wtile = wtiles[g // R]
                        gg = g % R
                        for h in range(2):
                            lhs = wtile[:, gg * A + h * 128:
                                        gg * A + h * 128 + 128]
                            for v in range(3):
                                nc.tensor.matmul(
                                    uacc[v][h][:], lhsT=lhs,
                                    rhs=xts[v][:, g * D:(g + 1) * D],
                                    start=(b == 0 and g == 0), stop=last)
                    continue
                for v in range(3):
                    engs[qi % nq].dma_start(xts[v][:], xrs[v][b])
                    qi += 1
                for k in range(wpb):
                    wtile = pw.tile([P, R * A], BF16, name="w", tag="w")
                    engs[qi % nq].dma_start(wtile[:], wtr[b][k])
                    qi += 1
                    wtiles.append(wtile)
                if not matmul:
                    continue
                for g in range(r_):
                    first = (b == 0 and g == 0)
                    last = (b == NB - 1 and g == r_ - 1)
                    wtile = wtiles[g // R]
                    gg = g % R
                    for h in range(2):
                        lhs = wtile[:, gg * A + h * 128:
                                    gg * A + h * 128 + 128]
                        for v in range(3):
                            nc.tensor.matmul(
                                uacc[v][h][:],
                                lhsT=lhs,
                                rhs=xts[v][:, g * D:(g + 1) * D],
                                start=first, stop=last)
            if not matmul:
                tok = small.tile([P, FW], BF16, name="tok", tag="tok")
                nc.vector.tensor_copy(tok[:], stash[-1][0][:, 0:FW])
                nc.sync.dma_start(outr[0], tok[:])
                return
            # drain PSUM -> cc_in; fold u1 into the diff GEMMs:
            #   col(h,0) = u1[h];  col(h,v) = u1[h] + ud_v[h]  (v = 1, 2)
            # u1 copies on ACT (parallel with DVE adds); each h-half's
            # staging DMA issues as soon as that half is drained
            ccin_d = pdram.tile([128, 6 * D], BF16, name=f"ccin{r}",
                                tag=f"ccin{r}")
            for h in range(2):
                c0 = (h * 3) * D
                nc.scalar.activation(cc_in[:, c0:c0 + D], uacc[0][h][:],
                                     mybir.ActivationFunctionType.Copy)
                nc.vector.tensor_add(cc_in[:, c0 + D:c0 + 2 * D],
                                     cc_in[:, c0:c0 + D], uacc[1][h][:])
                nc.vector.tensor_add(cc_in[:, c0 + 2 * D:c0 + 3 * D],
                                     cc_in[:, c0:c0 + D], uacc[2][h][:])
                eng = nc.sync if h == 0 else nc.scalar
                eng.dma_start(ccin_d[:, c0:c0 + 3 * D],
                              cc_in[:, c0:c0 + 3 * D])

        # ---------------- all-reduce the GEMM partials (bf16) ---------------
        ccout_d = pdram.tile([128, 6 * D], BF16, name=f"ccout{r}",
                             tag=f"ccout{r}")
        if collective:
            nc.gpsimd.collective_compute(
                "AllReduce", mybir.AluOpType.add,
                replica_groups=[list(range(n_cores))],
                ins=[ccin_d.opt()], outs=[ccout_d.opt()])
        else:
            nc.sync.dma_start(ccout_d[:], ccin_d[:])
        t_tanh = cc_in
        # readback + tanh split by h-half on parallel queues so the score
        # chain starts as soon as the first half lands
        for h in range(2):
            c0 = (h * 3) * D
            eng = nc.sync if h == 0 else nc.scalar
            eng.dma_start(t_tanh[:, c0:c0 + 3 * D], ccout_d[:, c0:c0 + 3 * D])

        # ---------------- tanh, scores, softmax, beta ----------------------
        for h in range(2):
            c0 = (h * 3) * D
            nc.scalar.activation(t_tanh[:, c0:c0 + 3 * D],
                                 t_tanh[:, c0:c0 + 3 * D], Tanh)

        with (
            tc.tile_pool(name=f"ps_{r}", bufs=1, space="PSUM") as ps,
        ):
            # s = h_n @ t for all 3 views at once; (h,v,d) layout makes the
            # v*d columns for a fixed h contiguous (2 x 512 + 2 x 256 free)
            s01 = ps.tile([1, 2 * D], FP32, name="s01", tag="s01")
            s2 = ps.tile([1, D], FP32, name="s2", tag="s2")
            for h in range(2):
                base = h * 3 * D
                nc.tensor.matmul(s01[:], lhsT=hn_sb[:, h:h + 1],
                                 rhs=t_tanh[:, base:base + 2 * D],
                                 start=(h == 0), stop=(h == 1))
                nc.tensor.matmul(s2[:], lhsT=hn_sb[:, h:h + 1],
                                 rhs=t_tanh[:, base + 2 * D:base + 3 * D],
                                 start=(h == 0), stop=(h == 1))
            e = small.tile([1, 3 * D], FP32, name="e", tag="e")
            nc.scalar.activation(e[:, 0:2 * D], s01[:], Exp)
            nc.scalar.activation(e[:, 2 * D:3 * D], s2[:], Exp)
            ssum = ps.tile([1, D], FP32, name="ssum", tag="ssum")
            nc.vector.tensor_add(ssum[:], e[:, 0:D], e[:, D:2 * D])
            nc.vector.tensor_add(ssum[:], ssum[:], e[:, 2 * D:3 * D])
            rinv = ps.tile([1, D], FP32, name="rinv", tag="rinv")
            nc.vector.reciprocal(rinv[:], ssum[:])
            # beta_rep [1, 4*D] = [b2 | b2 | b3 | b3]
            brep = small.tile([1, 4 * D], BF16, name="brep", tag="brep")
            for j, v in ((0, 1), (1, 1), (2, 2), (3, 2)):
                nc.vector.tensor_mul(brep[:, j * D:(j + 1) * D],
                                     e[:, v * D:(v + 1) * D], rinv[:])
        # broadcast beta_rep across partitions, then widen to full FW by
        # doubling copies (4x-mode bf16 SBUF copies)
        B23 = small.tile([128, 4 * D], BF16, name="B23", tag="B23")
        with (
            tc.tile_pool(name=f"pB_{r}", bufs=2, space="PSUM") as pB,
        ):
            for v in range(2):
                Bp = pB.tile([128, 2 * D], FP32, name=f"Bp{v}", tag="Bp")
                nc.tensor.matmul(Bp[:], lhsT=ones_sb[:],
                                 rhs=brep[:, v * 2 * D:(v + 1) * 2 * D],
                                 start=True, stop=True)
                nc.vector.tensor_copy(B23[:, v * 2 * D:(v + 1) * 2 * D],
                                      Bp[:])
        SL = 5 * D               # phase-2 slice width
        if p2pe:
            Brv2h = []
            for v in range(2):
                t = small.tile([128, FW // 2], BF16, name=f"Bh{v}",
                               tag=f"Bh{v}")
                nc.vector.tensor_copy(t[:, 0:2 * D],
                                      B23[:, v * 2 * D:(v + 1) * 2 * D])
                w = 2 * D
                while w < FW // 2:
                    c = min(w, FW // 2 - w)
                    nc.vector.tensor_copy(t[:, w:w + c], t[:, 0:c])
                    w += c
                Brv2h.append(t[0:P, :])
        Brv = []
        for v in range(2):
            t = small.tile([128, SL], BF16, name=f"Brv{v}", tag=f"Brv{v}")
            nc.vector.tensor_copy(t[:, 0:2 * D],
                                  B23[:, v * 2 * D:(v + 1) * 2 * D])
            w = 2 * D
            while w < SL:
                c = min(w, SL - w)
                nc.vector.tensor_copy(t[:, w:w + c], t[:, 0:c])
                w += c
            Brv.append(t[0:P, :])
        B2op, B3op = Brv

        # ---------------- phase 2: out = x1 + b2*d2 + b3*d3 -----------------
        if not phase2:
            m = min(FW, 6 * D)
            nc.sync.dma_start(outr[0][:, 0:m], t_tanh[0:P, 0:m])
            return
        if p2pe:
            HWD = FW // 2        # DVE mult width (half batch)
            PEC = 512            # PE accumulation chunk (one PSUM bank)
            ncp = HWD // PEC
            with (
                tc.tile_pool(name=f"pm2_{r}", bufs=2) as pm2,
                tc.tile_pool(name=f"pm3_{r}", bufs=2) as pm3,
                tc.tile_pool(name=f"pps_{r}", bufs=6, space="PSUM") as pps,
                tc.tile_pool(name=f"pot_{r}", bufs=2) as pot,
            ):
                B2h, B3h = Brv2h
                k = 0
                for b in range(NB):
                    x1t, d2t, d3t = stash[b]
                    for hf in range(2):
                        sl = slice(hf * HWD, (hf + 1) * HWD)
                        m2 = pm2.tile([P, HWD], BF16, name="m2", tag="m2")
                        m3 = pm3.tile([P, HWD], BF16, name="m3", tag="m3")
                        nc.vector.tensor_mul(m2[:], d2t[:, sl], B2h)
                        nc.vector.tensor_mul(m3[:], d3t[:, sl], B3h)
                        ot = pot.tile([P, HWD], BF16, name="o", tag="o")
                        for c in range(ncp):
                            cs = slice(c * PEC, (c + 1) * PEC)
                            xcs = slice(hf * HWD + c * PEC,
                                        hf * HWD + (c + 1) * PEC)
                            ps_c = pps.tile([P, PEC], FP32, name="pp",
                                            tag="pp")
                            nc.tensor.matmul(ps_c[:], lhsT=ident_sb[:],
                                             rhs=x1t[:, xcs], start=True,
                                             stop=False)
                            nc.tensor.matmul(ps_c[:], lhsT=ident_sb[:],
                                             rhs=m2[:, cs], start=False,
                                             stop=False)
                            nc.tensor.matmul(ps_c[:], lhsT=ident_sb[:],
                                             rhs=m3[:, cs], start=False,
                                             stop=True)
                            ceng = (nc.scalar.activation
                                    if c % 5 in (0, 2, 4) else None)
                            if c % 5 in (0, 2, 4):
                                nc.scalar.activation(
                                    ot[:, cs], ps_c[:],
                                    mybir.ActivationFunctionType.Copy)
                            else:
                                nc.vector.tensor_copy(ot[:, cs], ps_c[:])
                        deng = nc.sync if k % 2 == 0 else nc.scalar
                        deng.dma_start(outr[b][:, sl], ot[:])
                        k += 1
            return
        NSL = FW // SL           # slices per batch
        nsl_tot = NB * NSL
        with (
            tc.tile_pool(name=f"pout_{r}", bufs=3) as pout,
            tc.tile_pool(name=f"ptmp_{r}", bufs=2) as ptmp,
        ):
            pool_set = (set() if pool_batches == 0 else
                        {1 + i * (nsl_tot // pool_batches)
                         for i in range(pool_batches)})
            k = 0
            for b in range(NB):
                x1t, d2t, d3t = stash[b]
                for q in range(NSL):
                    sl = slice(q * SL, (q + 1) * SL)
                    ot = pout.tile([P, SL], BF16, name="o", tag="o")
                    m2 = ptmp.tile([P, SL], BF16, name="m2", tag="m2")
                    eng = nc.gpsimd if k in pool_set else nc.vector
                    eng.tensor_mul(ot[:], d3t[:, sl], B3op)
                    eng.tensor_mul(m2[:], d2t[:, sl], B2op)
                    eng.tensor_add(ot[:], ot[:], m2[:])
                    eng.tensor_add(ot[:], ot[:], x1t[:, sl])
                    if k == nsl_tot - 1:
                        hw2 = SL // 2
                        nc.sync.dma_start(outr[b][:, q * SL:q * SL + hw2],
                                          ot[:, 0:hw2])
                        nc.scalar.dma_start(
                            outr[b][:, q * SL + hw2:(q + 1) * SL],
                            ot[:, hw2:SL])
                    else:
                        deng = nc.sync if k % 2 == 0 else nc.scalar
                        deng.dma_start(outr[b][:, sl], ot[:])
                    k += 1


def build_bass(n_cores=N_CORES, collective=True, repeat=1, phase2=True,
               matmul=True, pool_batches=4, nq=3, r_=20, p2pe=False,
               **_ignored):
    nc = bacc.Bacc("TRN2", target_bir_lowering=False, debug=False,
                   num_devices=n_cores)

    x1 = nc.dram_tensor("x1", [N_LOC, D], BF16, kind="ExternalInput")
    d2 = nc.dram_tensor("d2", [N_LOC, D], BF16, kind="ExternalInput")
    d3 = nc.dram_tensor("d3", [N_LOC, D], BF16, kind="ExternalInput")
    wt = nc.dram_tensor("wt", [N_LOC, A], BF16, kind="ExternalInput")
    hnt = nc.dram_tensor("hnt", [A, 1], BF16, kind="ExternalInput")
    idn = nc.dram_tensor("idn", [P, P], BF16, kind="ExternalInput")
    out = nc.dram_tensor("out", [N_LOC, D], BF16, kind="ExternalOutput")

    with tile.TileContext(nc) as tc:
        with (
            tc.tile_pool(name="smallg", bufs=1) as smallg,
            tc.tile_pool(name="pdram", bufs=1, space="DRAM") as pdram,
        ):
            x1r = x1.ap().rearrange("(b p r) d -> b p (r d)", p=P, r=r_)
            d2r = d2.ap().rearrange("(b p r) d -> b p (r d)", p=P, r=r_)
            d3r = d3.ap().rearrange("(b p r) d -> b p (r d)", p=P, r=r_)
            wtr = wt.ap().rearrange("(b p k rr) a -> b k p (rr a)",
                                    p=P, k=r_ // R, rr=R)
            outr = out.ap().rearrange("(b p r) d -> b p (r d)", p=P, r=r_)
            xrs = (x1r, d2r, d3r)

            # h_n laid out [a_half(128 partitions), h(2)]
            hn_sb = smallg.tile([128, 2], BF16, tag="hn")
            nc.sync.dma_start(hn_sb[:, :],
                              hnt.ap().rearrange("(h a) o -> a (h o)", h=2))
            ones_sb = smallg.tile([1, 128], BF16, tag="ones")
            nc.vector.memset(ones_sb[:], 1.0)
            ident_sb = smallg.tile([P, P], BF16, tag="ident")
            nc.scalar.dma_start(ident_sb[:], idn.ap())

            for rep in range(repeat):
                _emit_iteration(nc, tc, rep, xrs, wtr, outr, hn_sb, ones_sb,
                                pdram, n_cores, collective, phase2, matmul,
                                pool_batches, nq, r_, ident_sb, p2pe)

    nc.compile()
    return nc


_NC_CACHE = {}


def _get_nc():
    if "nc" not in _NC_CACHE:
        _NC_CACHE["nc"] = build_bass()
    return _NC_CACHE["nc"]


def kernel(x1, x2, x3, W, h_n):
    import ml_dtypes
    bf16 = ml_dtypes.bfloat16

    x1 = np.asarray(x1, dtype=np.float32)
    x2 = np.asarray(x2, dtype=np.float32)
    x3 = np.asarray(x3, dtype=np.float32)
    W = np.asarray(W, dtype=np.float32)
    h_n = np.asarray(h_n, dtype=np.float32)

    x1b = x1.astype(bf16)
    d2b = (x2 - x1b.astype(np.float32)).astype(bf16)
    d3b = (x3 - x1b.astype(np.float32)).astype(bf16)

    hnt = np.ascontiguousarray(h_n.reshape(-1)[:, None]).astype(bf16)
    idn = np.eye(N_LOC // 100, dtype=np.float32).astype(bf16)
    in_maps = []
    for c in range(N_CORES):
        sl = slice(c * N_LOC, (c + 1) * N_LOC)
        in_maps.append({
            "x1": np.ascontiguousarray(x1b[sl]),
            "d2": np.ascontiguousarray(d2b[sl]),
            "d3": np.ascontiguousarray(d3b[sl]),
            "wt": np.ascontiguousarray(W[:, sl].T).astype(bf16),
            "hnt": hnt,
            "idn": idn,
        })

    nc = _get_nc()
    res = run_bass_kernel_spmd(nc, in_maps, core_ids=list(range(N_CORES)))
    return np.concatenate(
        [res.results[c]["out"].astype(np.float32) for c in range(N_CORES)],
        axis=0)
